# revision 15
# baseline (speedup 1.0000x reference)
"""Blinn-Phong env-map shader on 8 Trainium2 NeuronCores (Bass/Tile).

kernel(**inputs) takes the FULL inputs and returns
(colors (2,256,256,3) f32, pixel_normals (2,256,256,3) f32).

Sharding: faces split 8 ways for the mesh stage (vertex normals via
one-hot-matmul binning + AllReduce; per-face attrs AllGathered); pixels
split 8 ways for shading. Host only reshapes/pads/slices inputs and
reassembles outputs.

Math vs reference:
- specular: clip(n.Hv,0,1)^s with Hv=normalize(view+L) is computed as
  exp(s*ln(relu(A')) - (s/2)*ln(hh)), A' = n.v + n.L (K=4 fp32 matmul),
  hh = |v+L|^2 built from per-component half-vectors hv_d = v_d + L_d
  (K=4 fp16 hi/lo-split outer-sum matmuls; exact to ~2e-7, avoiding the
  catastrophic cancellation of 2+2*v.L for near-antipodal v,L), then
  hh = sum of fp32 squares. Upper clip dropped (overshoot ~1e-4 under ^s).
- diffuse: clip(n.L,0,1) -> relu(n.L) via bf16 matmul, upper clip dropped.
- vertex-normal binning matmuls run in fp32r (~2^-13 rel, fine for
  normals that are later normalized).
"""

import numpy as np

V = 20000
VP = 20480
F = 40000
FP_FACES = 40960
FS = 5120
H = W = 256
NPIX = 65536
P = 8192
N_CORES = 8
J = 256
B = 2


def _wrap_idx(idx):
    n = len(idx)
    w = idx.reshape(n // 16, 16).T.astype(np.int16)
    return np.tile(w, (8, 1))


def _build_program(shin, kd, ks, cam):
    import concourse.bass as bass
    import concourse.tile as tile
    from concourse import bacc, mybir
    from concourse.library_config import mlp as mlp_lib
    from concourse.masks import make_identity
    from concourse.tile_rust import add_dep_helper

    FP = mybir.dt.float32
    FPR = mybir.dt.float32r
    BF = mybir.dt.bfloat16
    F16 = mybir.dt.float16
    I16 = mybir.dt.int16
    I32 = mybir.dt.int32
    AF = mybir.ActivationFunctionType
    ALU = mybir.AluOpType

    bp_norm = (shin + 2.0) / (4.0 * (2.0 - np.exp(-shin / 2.0)))

    def last_inst(nc):
        return nc.main_func.blocks[-1].instructions[-1]

    nc = bacc.Bacc("TRN2", target_bir_lowering=False, debug=False, num_devices=N_CORES)

    verts_pad = nc.dram_tensor("verts_pad", [VP, 64], FP, kind="ExternalInput")
    faces_pm = nc.dram_tensor("faces_pm", [128, 120], I32, kind="ExternalInput")
    widx_mesh = nc.dram_tensor("widx_mesh", [128, 960], I16, kind="ExternalInput")
    widx_pix = nc.dram_tensor("widx_pix", [128, 512], I16, kind="ExternalInput")
    pixparity = nc.dram_tensor("pixparity", [128, 64], FP, kind="ExternalInput")
    bary_in = nc.dram_tensor("bary_pp", [128, 192], FP, kind="ExternalInput")
    ld_in = nc.dram_tensor("ld_in", [B, J, 3], FP, kind="ExternalInput")
    lc_in = nc.dram_tensor("lc_in", [B, J, 3], FP, kind="ExternalInput")

    colors0 = nc.dram_tensor("colors0", [3, P], FP, kind="ExternalOutput")
    colors1 = nc.dram_tensor("colors1", [3, P], FP, kind="ExternalOutput")
    pn_out = nc.dram_tensor("pn_out", [3, P], FP, kind="ExternalOutput")
    colors_out = [colors0, colors1]

    with tile.TileContext(nc) as tc:
        with (
            tc.tile_pool(name="sb", bufs=1) as sb,
            tc.tile_pool(name="scp", bufs=2) as scp,
            tc.tile_pool(name="dr", bufs=1, space="DRAM") as dr,
        ):
            # ---- constants (standard-lib gpsimd iota BEFORE mlp load) ----
            iq32 = sb.tile([128, 160], I32)
            nc.gpsimd.iota(iq32[:], pattern=[[1, 160]], base=0, channel_multiplier=0)
            i_iota1 = last_inst(nc)
            ir32 = sb.tile([128, 128], I32)
            nc.gpsimd.iota(ir32[:], pattern=[[1, 128]], base=0, channel_multiplier=0)
            i_iota2 = last_inst(nc)

            nc.gpsimd.load_library(mlp_lib)
            ld_lib = last_inst(nc)
            add_dep_helper(ld_lib, i_iota1, reason="lib switch after iota")
            add_dep_helper(ld_lib, i_iota2, reason="lib switch after iota")

            def mlp_op():
                add_dep_helper(last_inst(nc), ld_lib, reason="needs mlp lib")

            iqf = sb.tile([128, 160], FP)
            nc.vector.tensor_copy(iqf[:], iq32[:])
            irf = sb.tile([128, 128], FP)
            nc.vector.tensor_copy(irf[:], ir32[:])
            ident = sb.tile([128, 128], FP)
            make_identity(nc, ident[:])

            # ---- light matrices ----
            ldT = sb.tile([4, 512], FP)
            nc.vector.memset(ldT[:], 1.0)
            nc.sync.dma_start(
                ldT[0:3, :].rearrange("c (b jt j) -> c b jt j", b=B, jt=2),
                ld_in.ap().rearrange("b (jt j) c -> c b jt j", jt=2),
            )
            lcT = sb.tile([128, 12], FP)
            nc.sync.dma_start(
                lcT[:].rearrange("j (b jt c) -> j b jt c", b=B, jt=2),
                lc_in.ap().rearrange("b (jt j) c -> j b jt c", jt=2),
            )
            lcD = sb.tile([128, 12], BF)
            nc.vector.tensor_scalar(
                out=lcD[:], in0=lcT[:], scalar1=float(kd), scalar2=None, op0=ALU.mult
            )
            lcS = sb.tile([128, 12], BF)
            nc.vector.tensor_scalar(
                out=lcS[:],
                in0=lcT[:],
                scalar1=float(bp_norm * ks),
                scalar2=None,
                op0=ALU.mult,
            )

            # ---- pixel inputs ----
            baryt = sb.tile([128, 64, 3], FP)
            nc.sync.dma_start(baryt[:], bary_in.ap().rearrange("p (s c) -> p s c", c=3))
            part = sb.tile([128, 64], FP)
            nc.sync.dma_start(part[:], pixparity.ap())
            wpix = sb.tile([128, 512], I16)
            nc.sync.dma_start(wpix[:], widx_pix.ap())
            wmesh = sb.tile([128, 960], I16)
            nc.sync.dma_start(wmesh[:], widx_mesh.ap())
            fpm = sb.tile([128, 40, 3], I32)
            nc.sync.dma_start(fpm[:], faces_pm.ap().rearrange("p (s k) -> p s k", k=3))

            # =======================================================
            # Mesh stage
            # =======================================================
            plN = sb.tile([4, 8192], FP)  # rows nx,ny,nz,nv
            # view in fp16 hi/lo split: block d at partitions 32d..32d+4,
            # rows (1, 1, vhi_d, vlo_d); hv_d = Lhi+Llo+vhi+vlo via K=4
            # fp16 matmul (PSUM fp32 reconstructs v+L to ~2e-7).
            plVhl = sb.tile([68, 8192], F16)
            with (
                tc.tile_pool(name="msb", bufs=1) as msb,
                tc.tile_pool(name="mps", bufs=1, space="PSUM") as mps,
                tc.tile_pool(name="tps", bufs=1, space="PSUM") as tps,
            ):
                # M1: corner verts; corner i = k*5120 + (s'*128+p)
                vcorn = msb.tile([128, 120, 64], FP, tag="corn")
                for c in range(4):
                    nc.gpsimd.dma_gather(
                        out_ap=vcorn[:, 30 * c : 30 * (c + 1), :],
                        in_ap=verts_pad.ap(),
                        idxs_ap=wmesh[:, 240 * c : 240 * (c + 1)],
                        num_idxs=3840,
                        num_idxs_reg=3840,
                        elem_size=64,
                        single_packet=False,
                    )
                    mlp_op()

                # M2: fn = cross(v1-v0, v2-v0)
                d1 = msb.tile([128, 40, 3], FP)
                nc.vector.tensor_tensor(
                    out=d1[:],
                    in0=vcorn[:, 40:80, 0:3],
                    in1=vcorn[:, 0:40, 0:3],
                    op=ALU.subtract,
                )
                d2 = msb.tile([128, 40, 3], FP)
                nc.vector.tensor_tensor(
                    out=d2[:],
                    in0=vcorn[:, 80:120, 0:3],
                    in1=vcorn[:, 0:40, 0:3],
                    op=ALU.subtract,
                )
                fn = msb.tile([128, 40, 3], FP)
                tmpa = msb.tile([128, 40], FP)
                for d in range(3):
                    u, v_ = (d + 1) % 3, (d + 2) % 3
                    nc.vector.tensor_tensor(
                        out=fn[:, :, d], in0=d1[:, :, u], in1=d2[:, :, v_], op=ALU.mult
                    )
                    nc.vector.tensor_tensor(
                        out=tmpa[:], in0=d1[:, :, v_], in1=d2[:, :, u], op=ALU.mult
                    )
                    nc.vector.tensor_tensor(
                        out=fn[:, :, d], in0=fn[:, :, d], in1=tmpa[:], op=ALU.subtract
                    )

                # M3: binning into vn_ps[r, (q,d)] (fp32r matmuls: one-hots
                # are exact; fn rounds to ~2^-13 rel, fine for normals)
                msk = msb.tile([128, 40, 3], I32)
                nc.vector.tensor_scalar(
                    out=msk[:], in0=fpm[:], scalar1=127, scalar2=None,
                    op0=ALU.bitwise_and,
                )
                rf32 = msb.tile([128, 40, 3], FP)
                nc.vector.tensor_copy(rf32[:], msk[:])
                nc.vector.tensor_scalar(
                    out=msk[:], in0=fpm[:], scalar1=7, scalar2=None,
                    op0=ALU.logical_shift_right,
                )
                qf32 = msb.tile([128, 40, 3], FP)
                nc.vector.tensor_copy(qf32[:], msk[:])

                vn_ps = mps.tile([128, 480], FP)
                nmm = 0
                for k in range(3):
                    for sp in range(40):
                        ohq = scp.tile([128, 160], FP, tag="ohq")
                        nc.vector.tensor_scalar(
                            out=ohq[:],
                            in0=iqf[:],
                            scalar1=qf32[:, sp, k : k + 1],
                            scalar2=None,
                            op0=ALU.is_equal,
                        )
                        bmat = scp.tile([128, 160, 3], FPR, tag="bmat")
                        nc.vector.tensor_tensor(
                            out=bmat[:],
                            in0=ohq[:].rearrange("p (q o) -> p q o", o=1)
                            .to_broadcast([128, 160, 3]),
                            in1=fn[:, sp : sp + 1, :].to_broadcast([128, 160, 3]),
                            op=ALU.mult,
                        )
                        ohr = scp.tile([128, 128], FPR, tag="ohr")
                        nc.vector.tensor_scalar(
                            out=ohr[:],
                            in0=irf[:],
                            scalar1=rf32[:, sp, k : k + 1],
                            scalar2=None,
                            op0=ALU.is_equal,
                        )
                        nc.tensor.matmul(
                            out=vn_ps[:],
                            lhsT=ohr[:],
                            rhs=bmat[:].rearrange("p q c -> p (q c)"),
                            start=(nmm == 0),
                            stop=(nmm == 119),
                        )
                        nmm += 1

                # M4: AllReduce partial vn
                vn_sb = msb.tile([128, 480], FP)
                nc.scalar.copy(out=vn_sb[:], in_=vn_ps[:])
                arin = dr.tile([128, 480], FP)
                arout = dr.tile([128, 480], FP, addr_space="Shared")
                nc.sync.dma_start(arin[:], vn_sb[:])
                nc.gpsimd.collective_compute(
                    "AllReduce",
                    ALU.add,
                    replica_groups=[list(range(N_CORES))],
                    ins=[arin.opt()],
                    outs=[arout.opt()],
                )
                mlp_op()

                vnfat = dr.tile([VP, 64], FP)
                nc.sync.dma_start(
                    vnfat[:].rearrange("(q r) c -> r q c", r=128)[:, :, 0:3],
                    arout[:].rearrange("r (q c) -> r q c", c=3),
                )

                # M5: corner normals (same indices)
                ncorn = msb.tile([128, 120, 64], FP, tag="corn")
                for c in range(4):
                    nc.gpsimd.dma_gather(
                        out_ap=ncorn[:, 30 * c : 30 * (c + 1), :],
                        in_ap=vnfat[:],
                        idxs_ap=wmesh[:, 240 * c : 240 * (c + 1)],
                        num_idxs=3840,
                        num_idxs_reg=3840,
                        elem_size=64,
                        single_packet=False,
                    )
                    mlp_op()

                # M6: per-face attr rows (32f rows: 0:9 pos, 9:18 normals,
                # rest garbage) -> AllGather -> pair-gatherable table
                agin = dr.tile([FS, 32], FP)
                for k in range(3):
                    nc.sync.dma_start(
                        agin[:].rearrange("(s p) c -> p s c", p=128)[
                            :, :, 3 * k : 3 * k + 3
                        ],
                        vcorn[:, 40 * k : 40 * (k + 1), 0:3],
                    )
                    nc.sync.dma_start(
                        agin[:].rearrange("(s p) c -> p s c", p=128)[
                            :, :, 9 + 3 * k : 12 + 3 * k
                        ],
                        ncorn[:, 40 * k : 40 * (k + 1), 0:3],
                    )
                agout = dr.tile([FP_FACES, 32], FP, addr_space="Shared")
                nc.gpsimd.collective_compute(
                    "AllGather",
                    ALU.bypass,
                    replica_groups=[list(range(N_CORES))],
                    ins=[agin.opt()],
                    outs=[agout.opt()],
                )
                mlp_op()

                # ============ pixel stage ============
                fa = msb.tile([128, 64, 18], FP)
                for c in range(4):
                    fpair = msb.tile([128, 16, 64], FP, tag="fpair")
                    nc.gpsimd.dma_gather(
                        out_ap=fpair[:],
                        in_ap=agout[:].rearrange("(u two) c -> u (two c)", two=2),
                        idxs_ap=wpix[:, 128 * c : 128 * (c + 1)],
                        num_idxs=2048,
                        num_idxs_reg=2048,
                        elem_size=64,
                        single_packet=False,
                    )
                    mlp_op()
                    fsel = msb.tile([128, 16, 18], FP, tag="fsel")
                    nc.vector.tensor_tensor(
                        out=fsel[:],
                        in0=fpair[:, :, 32:50],
                        in1=fpair[:, :, 0:18],
                        op=ALU.subtract,
                    )
                    nc.vector.tensor_tensor(
                        out=fsel[:],
                        in0=fsel[:],
                        in1=part[:, 16 * c : 16 * (c + 1)]
                        .rearrange("p (s o) -> p s o", o=1)
                        .to_broadcast([128, 16, 18]),
                        op=ALU.mult,
                    )
                    nc.vector.tensor_tensor(
                        out=fa[:, 16 * c : 16 * (c + 1), :],
                        in0=fsel[:],
                        in1=fpair[:, :, 0:18],
                        op=ALU.add,
                    )

                pos = msb.tile([128, 64, 3], FP)
                pnrm = msb.tile([128, 64, 3], FP)
                tmp3 = msb.tile([128, 64, 3], FP)
                for dst, base in ((pos, 0), (pnrm, 9)):
                    nc.vector.tensor_tensor(
                        out=dst[:],
                        in0=fa[:, :, base : base + 3],
                        in1=baryt[:, :, 0:1].to_broadcast([128, 64, 3]),
                        op=ALU.mult,
                    )
                    for kk in (1, 2):
                        nc.vector.tensor_tensor(
                            out=tmp3[:],
                            in0=fa[:, :, base + 3 * kk : base + 3 * kk + 3],
                            in1=baryt[:, :, kk : kk + 1].to_broadcast([128, 64, 3]),
                            op=ALU.mult,
                        )
                        nc.vector.tensor_tensor(
                            out=dst[:], in0=dst[:], in1=tmp3[:], op=ALU.add
                        )

                nn = msb.tile([128, 64], FP)
                sq3 = msb.tile([128, 64, 3], FP)
                rsn = msb.tile([128, 64], FP)

                def norm3(vec):
                    nc.vector.tensor_tensor(
                        out=sq3[:], in0=vec[:], in1=vec[:], op=ALU.mult
                    )
                    nc.vector.tensor_reduce(
                        out=nn[:].rearrange("p (s o) -> p s o", o=1),
                        in_=sq3[:],
                        axis=mybir.AxisListType.X,
                        op=ALU.add,
                    )
                    nc.vector.tensor_scalar(
                        out=nn[:], in0=nn[:], scalar1=1e-12, scalar2=None, op0=ALU.max
                    )
                    nc.scalar.activation(rsn[:], nn[:], AF.Sqrt)
                    nc.vector.reciprocal(out=rsn[:], in_=rsn[:])
                    nc.vector.tensor_tensor(
                        out=vec[:],
                        in0=vec[:],
                        in1=rsn[:].rearrange("p (s o) -> p s o", o=1).to_broadcast([128, 64, 3]),
                        op=ALU.mult,
                    )

                norm3(pnrm)
                view = msb.tile([128, 64, 3], FP)
                for d in range(3):
                    nc.vector.tensor_scalar(
                        out=view[:, :, d],
                        in0=pos[:, :, d],
                        scalar1=-1.0,
                        scalar2=float(cam[d]),
                        op0=ALU.mult,
                        op1=ALU.add,
                    )
                norm3(view)
                nv = msb.tile([128, 64], FP)
                nc.vector.tensor_tensor(
                    out=sq3[:], in0=pnrm[:], in1=view[:], op=ALU.mult
                )
                nc.vector.tensor_reduce(
                    out=nv[:].rearrange("p (s o) -> p s o", o=1),
                    in_=sq3[:],
                    axis=mybir.AxisListType.X,
                    op=ALU.add,
                )

                cblk = msb.tile([128, 64, 16], FP)
                nc.vector.memset(cblk[:], 1.0)
                nc.vector.tensor_copy(cblk[:, :, 0:3], pnrm[:])
                nc.vector.tensor_copy(cblk[:, :, 3], nv[:])
                vhi16 = msb.tile([128, 64, 3], F16)
                nc.vector.tensor_copy(vhi16[:], view[:])
                vhi32 = msb.tile([128, 64, 3], FP)
                nc.vector.tensor_copy(vhi32[:], vhi16[:])
                vlo32 = msb.tile([128, 64, 3], FP)
                nc.vector.tensor_tensor(
                    out=vlo32[:], in0=view[:], in1=vhi32[:], op=ALU.subtract
                )
                nc.vector.tensor_copy(
                    cblk[:, :, 4:16].rearrange("p s (d t) -> p s d t", t=4)[
                        :, :, :, 2:3
                    ],
                    vhi32[:].rearrange("p s (d o) -> p s d o", o=1),
                )
                nc.vector.tensor_copy(
                    cblk[:, :, 4:16].rearrange("p s (d t) -> p s d t", t=4)[
                        :, :, :, 3:4
                    ],
                    vlo32[:].rearrange("p s (d o) -> p s d o", o=1),
                )

                for grp in range(16):
                    ptN = tps.tile([4, 512], FP, tag="plTN")
                    ptV = [
                        tps.tile([4, 512], FP, name=f"ptV{dd}", tag=f"plTV{dd}")
                        for dd in range(3)
                    ]
                    for u in range(4):
                        s = grp * 4 + u
                        nc.tensor.transpose(
                            out=ptN[:, 128 * u : 128 * (u + 1)],
                            in_=cblk[:, s, 0:4],
                            identity=ident[:],
                        )
                        for dd in range(3):
                            nc.tensor.transpose(
                                out=ptV[dd][:, 128 * u : 128 * (u + 1)],
                                in_=cblk[:, s, 4 + 4 * dd : 8 + 4 * dd],
                                identity=ident[:],
                            )
                    nc.scalar.copy(out=plN[:, 512 * grp : 512 * (grp + 1)], in_=ptN[:])
                    for dd in range(3):
                        nc.scalar.copy(
                            out=plVhl[32 * dd : 32 * dd + 4, 512 * grp : 512 * (grp + 1)],
                            in_=ptV[dd][:],
                        )

            nc.sync.dma_start(pn_out.ap(), plN[0:3, :])

            # =======================================================
            # Light stage (pixel chunks x all-512-light groups)
            # spec = relu(n.(v+L))^s / |v+L|^s with hv built exactly via
            # K=2 outer-sum matmuls; hh = sum of fp32 squares.
            # =======================================================
            eps12 = sb.tile([128, 1], FP)
            nc.vector.memset(eps12[:], 1e-12)
            plN_bf = sb.tile([4, 8192], BF)
            nc.vector.tensor_copy(plN_bf[:], plN[:])
            ldT_bf = sb.tile([4, 512], BF)
            nc.vector.tensor_copy(ldT_bf[:], ldT[:])
            # ldhl: block d at partitions 32d..32d+4, rows (Lhi_d, Llo_d,
            # 1, 1) — mirrors plVhl so lhsT/rhs share base partitions.
            ldhl = sb.tile([68, 512], F16)
            nc.vector.memset(ldhl[:], 1.0)
            ldrow = sb.tile([1, 512], FP)
            lhi16 = sb.tile([1, 512], F16)
            lhi32 = sb.tile([1, 512], FP)
            llo16 = sb.tile([1, 512], F16)
            for dd in range(3):
                nc.sync.dma_start(ldrow[:], ldT[dd : dd + 1, :])
                nc.vector.tensor_copy(lhi16[:], ldrow[:])
                nc.vector.tensor_copy(lhi32[:], lhi16[:])
                nc.vector.tensor_tensor(
                    out=llo16[:], in0=ldrow[:], in1=lhi32[:], op=ALU.subtract
                )
                nc.sync.dma_start(ldhl[32 * dd : 32 * dd + 1, :], lhi16[:])
                nc.sync.dma_start(ldhl[32 * dd + 1 : 32 * dd + 2, :], llo16[:])

            with (
                tc.tile_pool(name="lps", bufs=2, space="PSUM") as lps,
                tc.tile_pool(name="hvp", bufs=1, space="PSUM") as hvp,
                tc.tile_pool(name="pcp", bufs=1, space="PSUM") as pcp,
                tc.tile_pool(name="lsb", bufs=1) as lsb,
            ):
                for b in range(B):
                    wd = [
                        lsb.tile([128, 8192], BF, name=f"wd{b}_{jt}", tag=f"wd{jt}")
                        for jt in range(2)
                    ]
                    ws_ = [
                        lsb.tile([128, 8192], BF, name=f"ws{b}_{jt}", tag=f"ws{jt}")
                        for jt in range(2)
                    ]
                    for jt in range(2):
                        g = 2 * b + jt
                        gsl = slice(128 * g, 128 * (g + 1))
                        gb1 = lsb.tile([128, 8192], F16, name=f"g{b}_{jt}", tag="gb")
                        for scnk in range(16):
                            cols = slice(512 * scnk, 512 * (scnk + 1))
                            pA = lps.tile([128, 512], FP, tag="pA")
                            nc.tensor.matmul(
                                out=pA[:], lhsT=ldT[0:4, gsl], rhs=plN[0:4, cols],
                                start=True, stop=True,
                            )
                            pD = lps.tile([128, 512], FP, tag="pD")
                            nc.tensor.matmul(
                                out=pD[:], lhsT=ldT_bf[0:3, gsl],
                                rhs=plN_bf[0:3, cols], start=True, stop=True,
                            )
                            hv = hvp.tile([128, 3, 512], FP, tag="hv")
                            for dd in range(3):
                                nc.tensor.matmul(
                                    out=hv[:, dd, :],
                                    lhsT=ldhl[32 * dd : 32 * dd + 4, gsl],
                                    rhs=plVhl[32 * dd : 32 * dd + 4, cols],
                                    start=True, stop=True,
                                )
                            sq = scp.tile([128, 3, 512], FP, tag="sq")
                            nc.scalar.activation(
                                sq[:].rearrange("p d x -> p (d x)"),
                                hv[:].rearrange("p d x -> p (d x)"),
                                AF.Square,
                            )
                            hhs = scp.tile([128, 512], FP, tag="hhs")
                            nc.vector.tensor_tensor(
                                out=hhs[:], in0=sq[:, 0, :], in1=sq[:, 1, :],
                                op=ALU.add,
                            )
                            nc.vector.tensor_tensor(
                                out=hhs[:], in0=hhs[:], in1=sq[:, 2, :], op=ALU.add
                            )
                            lu = scp.tile([128, 512], FP, tag="lu")
                            nc.scalar.activation(lu[:], hhs[:], AF.Ln, bias=eps12[:])
                            tre = scp.tile([128, 512], FP, tag="tre")
                            nc.vector.tensor_scalar(
                                out=tre[:], in0=pA[:], scalar1=0.0, scalar2=None,
                                op0=ALU.max,
                            )
                            lt = scp.tile([128, 512], FP, tag="lt")
                            nc.scalar.activation(lt[:], tre[:], AF.Ln)
                            nc.vector.scalar_tensor_tensor(
                                out=gb1[:, cols],
                                in0=lu[:],
                                scalar=-0.5,
                                in1=lt[:],
                                op0=ALU.mult,
                                op1=ALU.add,
                            )
                            nc.vector.tensor_scalar(
                                out=wd[jt][:, cols], in0=pD[:], scalar1=0.0,
                                scalar2=None, op0=ALU.max,
                            )

                        nc.scalar.activation(
                            ws_[jt][:], gb1[:], AF.Exp, scale=float(shin)
                        )

                    lcb = []
                    for qi, lct in enumerate((lcD, lcS)):
                        for jt in range(2):
                            t_ = lsb.tile(
                                [128, 3], BF, name=f"lcb{b}_{qi}_{jt}",
                                tag=f"lcb{qi}{jt}",
                            )
                            nc.vector.tensor_copy(
                                t_[:], lct[:, 3 * (2 * b + jt) : 3 * (2 * b + jt) + 3]
                            )
                            lcb.append((t_, qi, jt))
                    for scnk in range(16):
                        cols = slice(512 * scnk, 512 * (scnk + 1))
                        pc = pcp.tile([3, 512], FP, tag="pc")
                        for li, (t_, qi, jt) in enumerate(lcb):
                            wt = wd[jt] if qi == 0 else ws_[jt]
                            nc.tensor.matmul(
                                out=pc[:],
                                lhsT=t_[:],
                                rhs=wt[:, cols],
                                start=(li == 0),
                                stop=(li == 3),
                            )
                        cts = scp.tile([3, 512], FP, tag="cts")
                        nc.vector.tensor_copy(cts[:], pc[:])
                        nc.sync.dma_start(colors_out[b].ap()[:, cols], cts[:])

    nc.compile()
    return nc


_CACHE = {}
LAST_EXEC_NS = None


def kernel(
    verts,
    faces,
    pix_to_face,
    bary_coords,
    light_directions,
    light_colors,
    camera_position,
    shininess,
    kd,
    ks,
):
    from concourse import bass_utils

    bass_utils.upload_artifacts = lambda tmpdir: f"local:{tmpdir}"

    verts = np.asarray(verts, np.float32)
    faces = np.asarray(faces, np.int32)
    pix = np.asarray(pix_to_face, np.int32).reshape(NPIX)
    bary = np.asarray(bary_coords, np.float32).reshape(NPIX, 3)
    ld = np.asarray(light_directions, np.float32)
    lc = np.asarray(light_colors, np.float32)
    cam = np.asarray(camera_position, np.float32).reshape(3)
    shin = float(np.asarray(shininess))
    kdv = float(np.asarray(kd))
    ksv = float(np.asarray(ks))

    key = (shin, kdv, ksv, tuple(cam.tolist()))
    if key not in _CACHE:
        _CACHE[key] = _build_program(shin, kdv, ksv, cam)
    nc = _CACHE[key]

    verts_pad = np.zeros((VP, 64), np.float32)
    verts_pad[:V, 0:3] = verts
    faces_pad = np.zeros((FP_FACES, 3), np.int32)
    faces_pad[:F] = faces

    in_maps = []
    for c in range(N_CORES):
        fslice = faces_pad[c * FS : (c + 1) * FS]
        fpm = fslice.reshape(40, 128, 3).transpose(1, 0, 2).reshape(128, 120)
        midx = fslice.T.reshape(-1).astype(np.int64)
        wmesh = _wrap_idx(midx)

        pslice = pix[c * P : (c + 1) * P]
        bslice = bary[c * P : (c + 1) * P]
        pix_pp = pslice.reshape(64, 128).T
        bary_pp = bslice.reshape(64, 128, 3).transpose(1, 0, 2).reshape(128, 192)
        wpix = _wrap_idx((pslice >> 1).astype(np.int64))
        parity = (pix_pp & 1).astype(np.float32)

        in_maps.append(
            {
                "verts_pad": verts_pad,
                "faces_pm": np.ascontiguousarray(fpm, np.int32),
                "widx_mesh": wmesh,
                "widx_pix": wpix,
                "pixparity": parity,
                "bary_pp": np.ascontiguousarray(bary_pp, np.float32),
                "ld_in": ld,
                "lc_in": lc,
            }
        )

    import os

    if os.environ.get("BASS_SHADER_SIM") == "1":
        from concourse.bass_interp import MultiCoreSim

        sim = MultiCoreSim(
            nc,
            num_cores=N_CORES,
            trace=False,
            require_finite=False,
            require_nnan=False,
        )
        for c in range(N_CORES):
            for k2, v2 in in_maps[c].items():
                sim.cores[c].tensor(k2)[:] = v2
        sim.simulate(check_with_hw=False)

        class _R:
            results = [
                {
                    k2: np.array(sim.cores[c].tensor(k2))
                    for k2 in ("colors0", "colors1", "pn_out")
                }
                for c in range(N_CORES)
            ]
            exec_time_ns = None
            instructions_and_trace = None

        res = _R()
    else:
        trace = os.environ.get("BASS_SHADER_TRACE") == "1"
        res = bass_utils.run_bass_kernel_spmd(
            nc, in_maps, core_ids=list(range(N_CORES)), trace=trace
        )
    global LAST_EXEC_NS
    LAST_EXEC_NS = res.exec_time_ns
    if res.instructions_and_trace:
        globals()["LAST_INSTS"] = res.instructions_and_trace

    colors = np.zeros((B, NPIX, 3), np.float32)
    pn = np.zeros((NPIX, 3), np.float32)
    for c in range(N_CORES):
        r = res.results[c]
        colors[0, c * P : (c + 1) * P, :] = r["colors0"].T
        colors[1, c * P : (c + 1) * P, :] = r["colors1"].T
        pn[c * P : (c + 1) * P, :] = r["pn_out"].T

    colors = colors.reshape(B, H, W, 3)
    pn_full = np.broadcast_to(pn.reshape(1, H, W, 3), (B, H, W, 3)).copy()
    return colors, pn_full

